# revision 34
# baseline (speedup 1.0000x reference)
"""Trainium2 Bass kernel for the CrossEntropyMap loss.

Math (per batch row b of y_hat[B=64, T=64, G=128, G]):
    lse_b  = logsumexp(y_hat[b].reshape(-1))            # over T*G*G = 1M classes
    pick_b = sum_t y_hat[b, t, xi[b,t], yi[b,t]]        # xi/yi = round(coords*G)
    loss   = mean_b(T * lse_b - pick_b)

Sharding: data-parallel over batch, 8 rows per NeuronCore (32 MiB/core).

Device kernel (per core): stream the 8 rows as 18 chunks (mostly half-row
[128, 4096], tapered at both ends — see CHUNKS) and run one ACT pass per
chunk: exp(x + C_SHIFT) with accum_out giving the per-partition sums
S[p, c]. The constant shift is mathematically exact for logsumexp (it only
scales the partial sums); C_SHIFT=-16 keeps exp in range for |x| up to
~100. The 512 picked logits are gathered with indirect DMAs (f32, straight
from HBM). One PE matmul with a ones vector reduces [S | -picksum] over
partitions to a [19, 1] output per core; the host folds ln(), the shift and
the batch mean while unsharding (64 scalar lns total).

DMA strategy: a single DGE queue only sustains ~210-240 GB/s, but with both
HWDGE rings (sync = qSPDynamicHW, scalar = qActDynamicHW) streaming
concurrently the 16 SDMA engines saturate at ~425 GB/s aggregate (measured;
~26.6 GB/s per SBUF AXI port). Chunks alternate between the two rings so
both stay busy until the very end, and exps consume them in the same
alternating order so ACT tracks arrivals without head-of-line blocking. A
third SWDGE bulk stream or f32->bf16 in-flight cast does NOT raise the
ceiling (same 16 engines bind on the read side) — measured 425 GB/s either
way — so gpsimd only runs the idx load + pick gather.

Measured structural hazards this layout dodges (see CHUNKS/PREFILL notes):
the idx load's tiny descriptors stalling a bulk ring FIFO; a 9th in-flight
DMA recycling one of the 8 DMAHW semaphore lanes and head-of-line blocking
the issuing sequencer; both rings draining simultaneously and serializing
two full 3.7us exps after the stream end; ACT p-state throttling (1.0 vs
1.2 GHz run-to-run) making early exp start worthwhile.
"""

import sys

import numpy as np

try:
    import concourse.bacc as bacc
except ImportError:  # pragma: no cover - fallback for bare environments
    sys.path.insert(0, "/opt/trn_rl_repo")
    import concourse.bacc as bacc

import concourse.bass as bass
import concourse.tile as tile
from concourse import mybir
from concourse.bass_utils import run_bass_kernel_spmd

B, T, G = 64, 64, 128
N_CORES = 8
ROWS = B // N_CORES            # 8 batch rows per core
ROW_ELEMS = T * G * G          # 1_048_576 classes per row
P = 128
F = ROW_ELEMS // P             # 8192 elements per partition per row
HALVES = 2                     # chunks per row
FH = F // HALVES               # 4096 per chunk
N_CHUNKS = ROWS * HALVES       # 16
N_PER_CORE = ROWS * ROW_ELEMS  # 8_388_608 elements per core shard
PICKS = ROWS * T               # 512 gathered logits per core
PICK_F = PICKS // P            # 4 per partition
C_SHIFT = -16.0                # constant exp bias (exact for logsumexp)

_f32 = mybir.dt.float32
_bf16 = mybir.dt.bfloat16
_i32 = mybir.dt.int32
_EXP = mybir.ActivationFunctionType.Exp
_AXF = mybir.AxisListType.X
_ADD = mybir.AluOpType.add

# --- stream configuration ---------------------------------------------------
# 'sy' = sync HWDGE ring, 'sc' = scalar HWDGE ring. Measured: the 16 SDMA
# engines cap at ~425 GB/s aggregate (~26.6 GB/s per port) once >=2
# descriptor streams are in flight, while a single stream only sustains
# ~210-240 GB/s — so split the bytes EQUALLY between both HWDGE rings and
# keep both busy until the very end. A third SWDGE bulk stream does not
# raise the ceiling (same engines), so gpsimd only runs the pick gather.
#
# Chunk list: (row, eighth_start, n_eighths) in units of F/8 = 1024 elems
# per partition. Row 0 tapers IN (2x 1 MiB then 2 MiB) so the first exp can
# start ~2.5us earlier — this matters in runs where ACT is p-state throttled
# (measured 1.0 GHz instead of 1.2 GHz) and becomes the critical path. Rows
# 1-6 stream as half-row 2 MiB chunks; row 7 tapers OUT (one 1 MiB chunk per
# ring at the end) so the two queues' simultaneous drain doesn't leave a
# serialized 2x3.7us exp tail. Descriptors below 8 KiB/partition measurably
# slow the stream, so the taper stops at 1 MiB chunks.
# sy ring: [1M, 2M x7, 1M, 0.5M] = 16.5 MiB; sc ring: [2M x7, 1.5M] =
# 15.5 MiB. Chunk BOUNDARIES are offset between the rings (sy completes at
# odd MiB marks, sc at even) so mid-run arrivals alternate every ~4.7us and
# ACT pipelines cleanly. The sc ring (which also starts ~2.7us late — its
# first packets queue behind the ACT-table-load DMA on the qAct ring)
# carries 1 MiB less and ends with a single 1.5M chunk, draining slightly
# before sy; sy's short [1M, 0.5M] tail then streams solo (a lone ring
# speeds up to ~366 GB/s) with exps that overlap it — without this, both
# rings drain together and two full 3.7us exps serialize after the stream
# end. A/B-measured best of three tail layouts (105.0-105.7us fast-state
# vs 105.6-106.1). Listed in consumption (arrival) order:
CHUNKS = [
    (0, 0, 2),   # 0  sy  1M    mark 1
    (1, 4, 4),   # 1  sc  2M    mark 2
    (0, 2, 4),   # 2  sy  2M    mark 3
    (2, 4, 4),   # 3  sc  2M    mark 4
    (1, 0, 4),   # 4  sy  2M    mark 5
    (3, 4, 4),   # 5  sc  2M    mark 6
    (2, 0, 4),   # 6  sy  2M    mark 7
    (4, 4, 4),   # 7  sc  2M    mark 8
    (3, 0, 4),   # 8  sy  2M    mark 9
    (5, 4, 4),   # 9  sc  2M    mark 10
    (4, 0, 4),   # 10 sy  2M    mark 11
    (6, 4, 4),   # 11 sc  2M    mark 12
    (5, 0, 4),   # 12 sy  2M    mark 13
    (7, 4, 4),   # 13 sc  2M    mark 14
    (6, 0, 4),   # 14 sy  2M    mark 15
    (7, 1, 3),   # 15 sc  1.5M  mark 15.5 (sc drains)
    (0, 6, 2),   # 16 sy  1M    mark 16   (solo)
    (7, 0, 1),   # 17 sy  0.5M  mark 16.5 (solo)
]
N_CHUNKS_DEV = len(CHUNKS)                                      # 18
CHUNK_STREAM = ["sy", "sc", "sy", "sc", "sy", "sc", "sy", "sc", "sy", "sc",
                "sy", "sc", "sy", "sc", "sy", "sc", "sy", "sy"]
# exp consumption order = arrival order.
EXP_ORDER = list(range(N_CHUNKS_DEV))
# host mapping: per-row list of device columns to sum for that row's S
ROW_COLS = [[0, 2, 16], [4, 1], [6, 3], [8, 5], [10, 7], [12, 9], [14, 11],
            [17, 15, 13]]
# prefill 4 per ring: there are exactly 8 DMAHW completion-semaphore lanes,
# and a 9th in-flight DMA reuses lane 0 — its dispatch then WAITS for chunk
# 0 to complete, head-of-line blocking the issuing sequencer (measured: a
# 5th prefill dispatch on the ACT engine stalled exp0 by ~7us). Reissued
# chunks (after each exp) reuse lanes that completed 4 chunks ago — no wait.
PREFILL = {"sy": 4, "sc": 4}

_compiled_nc = None
LAST_RESULTS = None  # test hook: BassKernelResults of the last run


def build_nc():
    nc = bacc.Bacc("TRN2", target_bir_lowering=False, debug=False)
    y = nc.dram_tensor("y", [N_PER_CORE, 1], _f32, kind="ExternalInput")
    idx = nc.dram_tensor("idx", [P, PICK_F], _i32, kind="ExternalInput")
    out = nc.dram_tensor("out", [N_CHUNKS_DEV + 1, 1], _f32, kind="ExternalOutput")

    # row view [ROWS, 128, 8192]: partition p of row r holds elements
    # [r*1M + p*8192, +8192) — contiguous per partition, so any run of
    # eighths (1024 elems) is one contiguous span per partition.
    y_rows = y.ap().rearrange("(r p f) o -> r p (f o)", r=ROWS, p=P)
    FE = F // 8  # 1024 elems per eighth

    def chunk_ap(c):
        r, e0, n = CHUNKS[c]
        return y_rows[r][:, e0 * FE : (e0 + n) * FE]

    with tile.TileContext(nc) as tc:
        with (
            tc.tile_pool(name="xpool", bufs=sum(PREFILL.values())) as xpool,
            tc.tile_pool(name="small", bufs=1) as small,
            tc.tile_pool(name="psum", bufs=1, space="PSUM") as psum,
        ):
            engines = {"sy": nc.sync, "sc": nc.scalar}

            ones = small.tile([P, 1], _f32)
            nc.vector.memset(ones[:], 1.0)
            cbias = small.tile([P, 1], _f32)
            nc.vector.memset(cbias[:], C_SHIFT)
            # idx load goes on the SWDGE (gpsimd) ring: its 128x16B
            # descriptors are RMW-slow and would stall the sy ring's FIFO
            # for ~4us ahead of the first bulk chunk; on the gather ring it
            # is off the critical path entirely.
            idx_sb = small.tile([P, PICK_F], _i32)
            nc.gpsimd.dma_start(out=idx_sb[:], in_=idx.ap())

            # s_all[:, c] = per-partition sum of exp(chunk c); last col = -picksum
            s_all = small.tile([P, N_CHUNKS_DEV + 1], _f32)

            # per-stream chunk lists in consumption order
            stream_chunks = {s: [c for c in EXP_ORDER if CHUNK_STREAM[c] == s]
                             for s in ("sy", "sc")}
            next_issue = {s: 0 for s in stream_chunks}
            x_tiles = {}

            def issue_dma(s):
                i = next_issue[s]
                if i >= len(stream_chunks[s]):
                    return
                next_issue[s] = i + 1
                c = stream_chunks[s][i]
                w = CHUNKS[c][2] * (F // 8)
                xt = xpool.tile([P, FH], _f32, tag="x")
                engines[s].dma_start(out=xt[:, 0:w], in_=chunk_ap(c))
                x_tiles[c] = xt

            # prefill both rings in global chunk order so buffer rotation
            # matches consumption order
            for c in range(PREFILL["sy"] + PREFILL["sc"]):
                issue_dma(CHUNK_STREAM[c])

            # picked-logit gather on the otherwise idle SWDGE queue; data is
            # only needed at the final reduce. NOTE: must be one transfer per
            # column — the HW DGE consumes ONE offset per partition per
            # transfer (a single [128,4]-offset indirect DMA gathers wrong
            # values; measured rel err 2.8e-4 vs 2e-7).
            picked = small.tile([P, PICK_F], _f32)
            for j in range(PICK_F):
                nc.gpsimd.indirect_dma_start(
                    out=picked[:, j : j + 1],
                    out_offset=None,
                    in_=y.ap(),
                    in_offset=bass.IndirectOffsetOnAxis(
                        ap=idx_sb[:, j : j + 1], axis=0
                    ),
                )
            # s_all[:, -1] = -sum_j picked[p, j]
            nc.vector.tensor_reduce(
                out=s_all[:, N_CHUNKS_DEV : N_CHUNKS_DEV + 1], in_=picked[:],
                axis=_AXF, op=_ADD, negate=True,
            )

            # stream the chunks through ACT in arrival order
            et = small.tile([P, FH], _bf16, tag="e")
            for c in EXP_ORDER:
                xt = x_tiles.pop(c)
                w = CHUNKS[c][2] * (F // 8)
                nc.scalar.activation(
                    out=et[:, 0:w], in_=xt[:, 0:w], func=_EXP,
                    bias=cbias[:, 0:1], scale=1.0,
                    accum_out=s_all[:, c : c + 1],
                )
                issue_dma(CHUNK_STREAM[c])

            # acc[j] = sum_p s_all[p, j]  (19 chunk sums + -picksum)
            acc = psum.tile([N_CHUNKS_DEV + 1, 1], _f32, tag="acc")
            nc.tensor.matmul(
                out=acc[:], lhsT=s_all[:], rhs=ones[:], start=True, stop=True
            )
            res = small.tile([N_CHUNKS_DEV + 1, 1], _f32)
            nc.vector.tensor_copy(out=res[:], in_=acc[:])
            nc.sync.dma_start(out=out.ap(), in_=res[:])

    nc.compile()
    return nc


def make_in_maps(y_hat: np.ndarray, coords: np.ndarray):
    """Shard inputs across cores and build per-core gather indices."""
    y_hat = np.ascontiguousarray(y_hat, dtype=np.float32)
    coords = np.asarray(coords, dtype=np.float32)

    # Match jnp.round (round-half-to-even); np.round has identical semantics,
    # and coords * 128 is exact in f32 (power-of-two scale).
    xi = np.round(coords[:, :, 0] * np.float32(G)).astype(np.int64)  # (B, T)
    yi = np.round(coords[:, :, 1] * np.float32(G)).astype(np.int64)  # (B, T)
    t = np.arange(T, dtype=np.int64)[None, :]
    flat = t * (G * G) + xi * G + yi  # (B, T) element offset within row b

    in_maps = []
    for c in range(N_CORES):
        rows = slice(c * ROWS, (c + 1) * ROWS)
        shard = y_hat[rows].reshape(N_PER_CORE, 1)
        local = np.arange(ROWS, dtype=np.int64)[:, None] * ROW_ELEMS + flat[rows]
        idx = local.reshape(P, PICK_F).astype(np.int32)
        in_maps.append({"y": shard, "idx": idx})
    return in_maps


def kernel(y_hat: np.ndarray, coords: np.ndarray) -> np.ndarray:
    global _compiled_nc, LAST_RESULTS
    in_maps = make_in_maps(y_hat, coords)
    if _compiled_nc is None:
        _compiled_nc = build_nc()
    res = run_bass_kernel_spmd(
        _compiled_nc, in_maps, core_ids=list(range(N_CORES))
    )
    LAST_RESULTS = res
    total = 0.0
    for r in res.results:
        v = np.asarray(r["out"]).reshape(-1).astype(np.float64)
        negpick = v[N_CHUNKS_DEV]
        s_rows = np.array([v[cols].sum() for cols in ROW_COLS])
        total += T * float(np.log(s_rows).sum()) + negpick
    loss = total / B + T * (-C_SHIFT)
    return np.array(np.float32(loss))
